# revision 14
# baseline (speedup 1.0000x reference)
"""GroupedQueryAttention on 8 Trainium2 NeuronCores.

Problem (hardcoded): B=2, T=2048, DIM=4096, 32 q heads, 8 kv heads, hd=128.
  q = x @ Wq.T ; k,v = split(x @ Wkv.T) ; causal softmax(q k^T/sqrt(hd)) v ; out = o @ Wo.T

Sharding: hybrid data x tensor parallel over 8 cores.
  core c -> batch b = c//4, kv-head group j = c%4 (kv heads {2j,2j+1}, q heads {8j..8j+7}).
Per core:
  phase 1: QT[e,t], KT[dk,t], VT[dv,t] projections (weights pre-transposed on host,
           x pre-transposed on host; all matmul inputs bf16, PSUM f32).
  phase 2: flash-style causal attention per q head in scores-TRANSPOSED layout
           sT[k,q] = KT_tile.T @ QT  (so the AV matmul takes exp(sT) directly as the
           moving operand and V[t,dv] as stationary - no P transposes).
           k-tiles processed in groups of 2 sharing one [128,1024] PSUM tile so the
           exp activation runs as one wide call; causal masking via left-aligned
           -1e30 mask adds on the diagonal k-tiles before exp (masked lanes exp to 0).
           Softmax denominators via a ones[128,1] matmul (partition-dim reduction);
           1/den via the fast Newton-seed reciprocal on DVE.
  phase 3: AllGather oT over the 4-core batch group (bf16), then each core computes
           a 1024-row slice of outT = Wo @ oT, written as f32.
Host: sums nothing - output slices are disjoint; just transpose/concat.
"""

import sys

sys.path.insert(0, "/opt/trn_rl_repo")

import math

import numpy as np

import concourse.bass as bass
import concourse.bacc as bacc
import concourse.tile as tile
from concourse import mybir
from concourse.bass_utils import run_bass_kernel_spmd

B, T, DIM = 2, 2048, 4096
N_HEADS, N_KV, HD = 32, 8, 128
R = N_HEADS // N_KV  # 4
NCORES = 8
GROUPS = [[0, 1, 2, 3], [4, 5, 6, 7]]

HPC = 8  # q heads per core
KVPC = 2  # kv heads per core
EQ = HPC * HD  # 1024 q-proj out features per core
EKV = KVPC * HD  # 256 k (and v) out features per core
NT = T // 512  # 4 t-groups of 512
NC = DIM // 128  # 32 contraction tiles
NKB = T // 128  # 16 k-tiles per head

BF = mybir.dt.bfloat16
F32 = mybir.dt.float32
INV_SQRT_HD = 1.0 / math.sqrt(HD)
MASKW = 128 + 256 + 384 + 512  # packed left-aligned diag masks
DEN_PAIR = True  # pre-sum exp pairs on DVE so den needs 1 matmul per 2 k-tiles


def build():
    nc = bacc.Bacc("TRN2", num_devices=NCORES)

    # ---- external I/O (per-core data differs, program is SPMD-identical) ----
    xT = nc.dram_tensor("xT", [DIM, T], BF, kind="ExternalInput")  # x[b].T
    wallT = nc.dram_tensor("wallT", [DIM, EQ + 2 * EKV], BF, kind="ExternalInput")
    woT = nc.dram_tensor("woT", [DIM, EQ], BF, kind="ExternalInput")  # Wo[oc_slice,:].T
    maskL = nc.dram_tensor("maskL", [128, MASKW], F32, kind="ExternalInput")
    ident = nc.dram_tensor("ident", [128, 128], BF, kind="ExternalInput")
    trimm = nc.dram_tensor("trimm", [128, 256], BF, kind="ExternalInput")
    ones_in = nc.dram_tensor("ones_in", [128, 1], BF, kind="ExternalInput")
    out_part = nc.dram_tensor("out_part", [EQ, T], F32, kind="ExternalOutput")

    EALL = EQ + 2 * EKV  # 1536, 12 e-tiles: 8 Q, 2 K, 2 V
    NE = EALL // 128

    # left-aligned diag mask for jd: starts at MOFF[jd], width (jd+1)*128
    MOFF = [0, 128, 384, 768]

    with tile.TileContext(nc) as tc:
        with (
            tc.tile_pool(name="persist", bufs=1) as persist,
            tc.tile_pool(name="stream", bufs=10) as stream,
            tc.tile_pool(name="work", bufs=3) as work,
            tc.tile_pool(name="dram2", bufs=1, space="DRAM") as dram2,
        ):
            # ---------------- constants ----------------
            mask_sb = persist.tile([128, MASKW], F32)
            nc.sync.dma_start(out=mask_sb[:], in_=maskL[:, :])
            ident_sb = persist.tile([128, 128], BF)
            nc.sync.dma_start(out=ident_sb[:], in_=ident[:, :])
            trimm_sb = persist.tile([128, 256], BF)
            nc.sync.dma_start(out=trimm_sb[:], in_=trimm[:, :])
            ones_sb = persist.tile([128, 1], BF)
            nc.sync.dma_start(out=ones_sb[:], in_=ones_in[:, :])

            # persistent activations
            qt_sb = persist.tile([128, HPC * T], BF)  # QT: head h at cols [h*T,(h+1)*T)
            kt_sb = persist.tile([128, KVPC * T], BF)  # KT per kv head
            vt_sb = persist.tile([128, KVPC * T], BF)  # VT per kv head
            v_sb = persist.tile([128, KVPC * T], BF)  # V[t,dv]: tile (g,kb) at (g*16+kb)*128

            # per-head AllGather buffers
            og_in = []
            og_out = []
            for h in range(HPC):
                oin = dram2.tile([128, T], BF, name=f"og_in_{h}")
                oout = dram2.tile([4 * 128, T], BF, name=f"og_out_{h}")
                og_in.append(oin)
                og_out.append(oout)

            with (
                tc.tile_pool(name="wall_pool", bufs=1) as wall_pool,
                tc.tile_pool(name="psum_p1", bufs=2, space="PSUM") as psum_p1,
            ):
                # phase-1 weights: c-tile cb at cols [cb*EALL, (cb+1)*EALL)
                wall_sb = wall_pool.tile([128, NC * EALL], BF)

                def load_wall(cb):
                    nc.sync.dma_start(
                        out=wall_sb[:, cb * EALL:(cb + 1) * EALL],
                        in_=wallT[cb * 128:(cb + 1) * 128, :],
                    )

                # ---------------- phase 1: projections ----------------
                # e-tile order: K0 K1 V0 V1 first so attention deps clear early
                etile_order = [HPC, HPC + 1, HPC + 2, HPC + 3] + list(range(HPC))

                def etile_dst(e):
                    # e indexes [Q0..Q7, K0, K1, V0, V1]
                    if e < HPC:
                        return qt_sb[:, e * T:(e + 1) * T]
                    if e < HPC + KVPC:
                        g = e - HPC
                        return kt_sb[:, g * T:(g + 1) * T]
                    g = e - HPC - KVPC
                    return vt_sb[:, g * T:(g + 1) * T]

                for chunk in range(3):  # 3 chunks of 4 e-tiles
                    es = etile_order[chunk * 4:(chunk + 1) * 4]
                    for tg in range(NT):
                        if chunk == 0 and tg == 0:
                            load_wall(0)
                            load_wall(1)
                            load_wall(2)
                        accs = []
                        for i, e in enumerate(es):
                            acc = psum_p1.tile([128, 512], F32, tag=f"acc{i}")
                            accs.append(acc)
                        for cb in range(NC):
                            if chunk == 0 and tg == 0 and cb + 3 < NC:
                                load_wall(cb + 3)
                            xt_t = stream.tile([128, 512], BF, tag="xt")
                            nc.sync.dma_start(
                                out=xt_t[:],
                                in_=xT[cb * 128:(cb + 1) * 128,
                                       tg * 512:(tg + 1) * 512],
                            )
                            for i, e in enumerate(es):
                                nc.tensor.matmul(
                                    accs[i][:],
                                    wall_sb[:, cb * EALL + e * 128:
                                            cb * EALL + (e + 1) * 128],
                                    xt_t[:],
                                    start=(cb == 0),
                                    stop=(cb == NC - 1),
                                )
                        for i, e in enumerate(es):
                            nc.vector.tensor_copy(
                                etile_dst(e)[:, tg * 512:(tg + 1) * 512], accs[i][:]
                            )

                # V = VT.T per 128x128 tile (PE transpose-mode; psum dtype = input)
                for g in range(KVPC):
                    for kb in range(NKB):
                        tp = psum_p1.tile([128, 128], BF, tag="acc0")
                        nc.tensor.transpose(
                            tp[:],
                            vt_sb[:, g * T + kb * 128:g * T + (kb + 1) * 128],
                            ident_sb[:],
                        )
                        nc.vector.tensor_copy(
                            v_sb[:, (g * NKB + kb) * 128:(g * NKB + kb + 1) * 128],
                            tp[:],
                        )

            # wall_pool/psum_p1 released; phase 2/3 reuse that SBUF/PSUM space.
            with (
                tc.tile_pool(name="p23", bufs=1) as p23,
                tc.tile_pool(name="work2", bufs=3) as work2,
            ):
                oT_sb = p23.tile([128, HPC * T], BF)  # local oT: head h at [h*T,..)
                woT_sb = p23.tile([128, NC * EQ], BF)  # phase-3 lhsT tiles
                for cb in range(NC):
                    nc.sync.dma_start(
                        out=woT_sb[:, cb * EQ:(cb + 1) * EQ],
                        in_=woT[cb * 128:(cb + 1) * 128, :],
                    )

                # ---------------- phase 2: attention ----------------
                # per-k-tile pipeline, 5-deep sT PSUM so the exp latency never
                # gates the PE; den via a gpsimd/DVE adder tree (quad sums) so
                # the PE only runs one accumulating den matmul per 4 k-tiles.
                # PSUM: sT 5x1 + oT 2x1 + den 1x1 = 8 banks.
                with (
                    tc.tile_pool(name="ps_sT", bufs=5, space="PSUM") as ps_sT,
                    tc.tile_pool(name="ps_oT", bufs=2, space="PSUM") as ps_oT,
                    tc.tile_pool(name="ps_den", bufs=1, space="PSUM") as ps_den,
                ):
                    for h in range(HPC):
                        g = h // R  # local kv head
                        qt_h = qt_sb[:, h * T:(h + 1) * T]
                        kt_g = kt_sb[:, g * T:(g + 1) * T]
                        for tg in range(NT):
                            nkb = 4 * tg + 4  # causal: k-tiles 0..nkb-1
                            nq = nkb // 4
                            oT_acc = ps_oT.tile([128, 512], F32, tag="oT")
                            den_acc = ps_den.tile([1, 512], F32, tag="den")
                            exps = []
                            for kb in range(nkb):
                                sT = ps_sT.tile([128, 512], F32, tag="sT")
                                jd = kb - 4 * tg  # diag 128-subtile (if 0..3)
                                jstart = max(0, jd)
                                diag = 0 <= jd < 4
                                nc.tensor.matmul(
                                    sT[:],
                                    kt_g[:, kb * 128:(kb + 1) * 128],
                                    qt_h[:, tg * 512:(tg + 1) * 512],
                                    start=True,
                                    stop=not diag,
                                )
                                if diag:
                                    # causal mask on the PE: accumulate
                                    # tri[r,c] = sum_j [j<r] * (-1e30)[j==c]
                                    nc.tensor.matmul(
                                        sT[:, jd * 128:(jd + 1) * 128],
                                        trimm_sb[:, 0:128],
                                        trimm_sb[:, 128:256],
                                        start=False,
                                        stop=True,
                                        skip_group_check=True,
                                    )
                                expT = work2.tile([128, 512], BF, tag="expT",
                                                  bufs=8)
                                if jstart > 0:
                                    nc.gpsimd.memset(expT[:, :jstart * 128], 0.0)
                                nc.scalar.activation(
                                    expT[:, jstart * 128:],
                                    sT[:, jstart * 128:],
                                    mybir.ActivationFunctionType.Exp,
                                    scale=INV_SQRT_HD,
                                )
                                exps.append(expT)
                                nc.tensor.matmul(
                                    oT_acc[:],
                                    v_sb[:, (g * NKB + kb) * 128:
                                         (g * NKB + kb + 1) * 128],
                                    expT[:],
                                    start=(kb == 0),
                                    stop=(kb == nkb - 1),
                                    skip_group_check=True,
                                )
                                if kb % 4 == 3:
                                    qd = kb // 4
                                    p0 = work2.tile([128, 512], BF, tag="dp0",
                                                    bufs=3)
                                    p1t = work2.tile([128, 512], BF, tag="dp1",
                                                     bufs=3)
                                    nc.vector.tensor_tensor(
                                        p0[:], exps[4 * qd][:],
                                        exps[4 * qd + 1][:],
                                        mybir.AluOpType.add,
                                    )
                                    nc.vector.tensor_tensor(
                                        p1t[:], exps[4 * qd + 2][:],
                                        exps[4 * qd + 3][:],
                                        mybir.AluOpType.add,
                                    )
                                    quad = work2.tile([128, 512], BF, tag="dq",
                                                      bufs=3)
                                    nc.vector.tensor_tensor(
                                        quad[:], p0[:], p1t[:],
                                        mybir.AluOpType.add,
                                    )
                                    nc.tensor.matmul(
                                        den_acc[:],
                                        ones_sb[:],
                                        quad[:],
                                        start=(qd == 0),
                                        stop=(qd == nq - 1),
                                        skip_group_check=True,
                                    )
                            recip = work2.tile([1, 512], F32, tag="recip", bufs=2)
                            nc.vector.reciprocal_approx_fast(
                                out=recip[:], in_=den_acc[:]
                            )
                            recip_b = work2.tile([128, 512], F32, tag="recip_b", bufs=2)
                            nc.gpsimd.partition_broadcast(recip_b[:], recip[:])
                            nc.vector.tensor_tensor(
                                oT_sb[:, h * T + tg * 512:h * T + (tg + 1) * 512],
                                oT_acc[:],
                                recip_b[:],
                                mybir.AluOpType.mult,
                            )
                        # ship this head's oT and gather peers'
                        nc.sync.dma_start(
                            out=og_in[h][:], in_=oT_sb[:, h * T:(h + 1) * T]
                        )
                        nc.gpsimd.collective_compute(
                            "AllGather",
                            mybir.AluOpType.bypass,
                            replica_groups=GROUPS,
                            ins=[og_in[h].opt()],
                            outs=[og_out[h].opt()],
                        )

                # ---------------- phase 3: outT slice = WoT.T @ oT_full --------
                # global e-tile eb <-> global head H: rank r = eb//8, local hl = eb%8
                with tc.tile_pool(name="ps_out", bufs=2, space="PSUM") as ps_out:
                    for tg in range(NT):
                        for occ in range(2):  # oc chunks of 4
                            accs = []
                            for oi in range(4):
                                acc = ps_out.tile([128, 512], F32, tag=f"out{oi}")
                                accs.append(acc)
                            eb_avail = [rr * HPC + hh
                                        for hh in range(HPC) for rr in range(4)]
                            for ei, eb in enumerate(eb_avail):  # 32 global e-tiles
                                r, hl = eb // HPC, eb % HPC
                                rhs_t = work2.tile([128, 512], BF, tag="rhs", bufs=8)
                                nc.sync.dma_start(
                                    out=rhs_t[:],
                                    in_=og_out[hl][r * 128:(r + 1) * 128,
                                                   tg * 512:(tg + 1) * 512],
                                )
                                for oi in range(4):
                                    oc = occ * 4 + oi
                                    nc.tensor.matmul(
                                        accs[oi][:],
                                        woT_sb[:, eb * EQ + oc * 128:
                                               eb * EQ + (oc + 1) * 128],
                                        rhs_t[:],
                                        start=(ei == 0),
                                        stop=(ei == NC - 1),
                                    )
                            for oi in range(4):
                                oc = occ * 4 + oi
                                ev = work2.tile([128, 512], F32, tag="ev")
                                nc.vector.tensor_copy(ev[:], accs[oi][:])
                                nc.sync.dma_start(
                                    out=out_part[oc * 128:(oc + 1) * 128,
                                                 tg * 512:(tg + 1) * 512],
                                    in_=ev[:],
                                )
    nc.finalize()
    return nc


_NC_CACHE = None


def _get_nc():
    global _NC_CACHE
    if _NC_CACHE is None:
        _NC_CACHE = build()
    return _NC_CACHE


def _make_maskL():
    """Packed left-aligned diag masks: for jd in 0..3 a [128, (jd+1)*128] block.

    Column c of block jd lies in subtile j = c // 128 (offset cc = c % 128);
    causal allow (k-offset r) <= (q-offset cc) on the diagonal subtile j == jd,
    full mask for j < jd, untouched (0) beyond the block width.
    """
    blocks = []
    r = np.arange(128)[:, None]
    for jd in range(4):
        w = (jd + 1) * 128
        c = np.arange(w)[None, :]
        j = c // 128
        cc = c % 128
        allow = (j == jd) & (r <= cc)
        m = np.where(allow, 0.0, -1e30).astype(np.float32)
        blocks.append(m)
    return np.concatenate(blocks, axis=1)  # [128, 1280]


def kernel(x, Wq, Wkv, Wo):
    x = np.asarray(x, dtype=np.float32)
    Wq = np.asarray(Wq, dtype=np.float32)
    Wkv = np.asarray(Wkv, dtype=np.float32)
    Wo = np.asarray(Wo, dtype=np.float32)

    # host-side prep (transposes + bf16 casts)
    try:
        import ml_dtypes

        bf16 = ml_dtypes.bfloat16
    except ImportError:  # pragma: no cover
        import jax.numpy as jnp

        bf16 = jnp.bfloat16

    xT_b = [np.ascontiguousarray(x[b].T).astype(bf16) for b in range(B)]

    maskL = _make_maskL()
    ident = np.eye(128, dtype=np.float32).astype(bf16)
    jj = np.arange(128)
    ones_lower = (jj[:, None] < jj[None, :]).astype(np.float32)  # [j, r] = j < r
    diagneg = np.where(jj[:, None] == jj[None, :], -1e30, 0.0).astype(np.float32)
    trimm = np.concatenate([ones_lower, diagneg], axis=1).astype(bf16)
    ones = np.ones((128, 1), dtype=np.float32).astype(bf16)

    in_maps = []
    for c in range(NCORES):
        b, j = c // 4, c % 4
        wq_l = Wq[EQ * j:EQ * (j + 1), :]  # [1024, 4096]
        wk_l = Wkv[EKV * j:EKV * (j + 1), :]  # [256, 4096]
        wv_l = Wkv[N_KV * HD + EKV * j:N_KV * HD + EKV * (j + 1), :]
        wall = np.concatenate([wq_l, wk_l, wv_l], axis=0)  # [1536, 4096]
        wallT = np.ascontiguousarray(wall.T).astype(bf16)  # [4096, 1536]
        woT_l = np.ascontiguousarray(Wo[EQ * j:EQ * (j + 1), :].T).astype(bf16)
        in_maps.append(
            {
                "xT": xT_b[b],
                "wallT": wallT,
                "woT": woT_l,
                "maskL": maskL,
                "ident": ident,
                "trimm": trimm,
                "ones_in": ones,
            }
        )

    nc = _get_nc()
    res = run_bass_kernel_spmd(nc, in_maps, core_ids=list(range(NCORES)))

    out = np.empty((B, T, DIM), dtype=np.float32)
    for b in range(B):
        outT = np.concatenate(
            [res.results[b * 4 + j]["out_part"] for j in range(4)], axis=0
        )  # [4096, 2048]
        out[b] = outT.T
    return out


# revision 15
# speedup vs baseline: 1.0031x; 1.0031x over previous
"""GroupedQueryAttention on 8 Trainium2 NeuronCores.

Problem (hardcoded): B=2, T=2048, DIM=4096, 32 q heads, 8 kv heads, hd=128.
  q = x @ Wq.T ; k,v = split(x @ Wkv.T) ; causal softmax(q k^T/sqrt(hd)) v ; out = o @ Wo.T

Sharding: hybrid data x tensor parallel over 8 cores.
  core c -> batch b = c//4, kv-head group j = c%4 (kv heads {2j,2j+1}, q heads {8j..8j+7}).
Per core:
  phase 1: QT[e,t], KT[dk,t], VT[dv,t] projections (weights pre-transposed on host,
           x pre-transposed on host; all matmul inputs bf16, PSUM f32).
  phase 2: flash-style causal attention per q head in scores-TRANSPOSED layout
           sT[k,q] = KT_tile.T @ QT  (so the AV matmul takes exp(sT) directly as the
           moving operand and V[t,dv] as stationary - no P transposes).
           k-tiles processed in groups of 2 sharing one [128,1024] PSUM tile so the
           exp activation runs as one wide call; causal masking via left-aligned
           -1e30 mask adds on the diagonal k-tiles before exp (masked lanes exp to 0).
           Softmax denominators via a ones[128,1] matmul (partition-dim reduction);
           1/den via the fast Newton-seed reciprocal on DVE.
  phase 3: AllGather oT over the 4-core batch group (bf16), then each core computes
           a 1024-row slice of outT = Wo @ oT, written as f32.
Host: sums nothing - output slices are disjoint; just transpose/concat.
"""

import sys

sys.path.insert(0, "/opt/trn_rl_repo")

import math

import numpy as np

import concourse.bass as bass
import concourse.bacc as bacc
import concourse.tile as tile
from concourse import mybir
from concourse.bass_utils import run_bass_kernel_spmd

B, T, DIM = 2, 2048, 4096
N_HEADS, N_KV, HD = 32, 8, 128
R = N_HEADS // N_KV  # 4
NCORES = 8
GROUPS = [[0, 1, 2, 3], [4, 5, 6, 7]]

HPC = 8  # q heads per core
KVPC = 2  # kv heads per core
EQ = HPC * HD  # 1024 q-proj out features per core
EKV = KVPC * HD  # 256 k (and v) out features per core
NT = T // 512  # 4 t-groups of 512
NC = DIM // 128  # 32 contraction tiles
NKB = T // 128  # 16 k-tiles per head

BF = mybir.dt.bfloat16
F32 = mybir.dt.float32
INV_SQRT_HD = 1.0 / math.sqrt(HD)
MASKW = 128 + 256 + 384 + 512  # packed left-aligned diag masks
DEN_PAIR = True  # pre-sum exp pairs on DVE so den needs 1 matmul per 2 k-tiles


def build():
    nc = bacc.Bacc("TRN2", num_devices=NCORES)

    # ---- external I/O (per-core data differs, program is SPMD-identical) ----
    xT = nc.dram_tensor("xT", [DIM, T], BF, kind="ExternalInput")  # x[b].T
    wallT = nc.dram_tensor("wallT", [DIM, EQ + 2 * EKV], BF, kind="ExternalInput")
    woT = nc.dram_tensor("woT", [DIM, EQ], BF, kind="ExternalInput")  # Wo[oc_slice,:].T
    maskL = nc.dram_tensor("maskL", [128, MASKW], F32, kind="ExternalInput")
    ident = nc.dram_tensor("ident", [128, 128], BF, kind="ExternalInput")
    ones_in = nc.dram_tensor("ones_in", [128, 1], BF, kind="ExternalInput")
    out_part = nc.dram_tensor("out_part", [EQ, T], F32, kind="ExternalOutput")

    EALL = EQ + 2 * EKV  # 1536, 12 e-tiles: 8 Q, 2 K, 2 V
    NE = EALL // 128

    # left-aligned diag mask for jd: starts at MOFF[jd], width (jd+1)*128
    MOFF = [0, 128, 384, 768]

    with tile.TileContext(nc) as tc:
        with (
            tc.tile_pool(name="persist", bufs=1) as persist,
            tc.tile_pool(name="stream", bufs=10) as stream,
            tc.tile_pool(name="work", bufs=3) as work,
            tc.tile_pool(name="dram2", bufs=1, space="DRAM") as dram2,
        ):
            # ---------------- constants ----------------
            mask_sb = persist.tile([128, MASKW], F32)
            nc.sync.dma_start(out=mask_sb[:], in_=maskL[:, :])
            ident_sb = persist.tile([128, 128], BF)
            nc.sync.dma_start(out=ident_sb[:], in_=ident[:, :])
            ones_sb = persist.tile([128, 1], BF)
            nc.sync.dma_start(out=ones_sb[:], in_=ones_in[:, :])

            # persistent activations
            qt_sb = persist.tile([128, HPC * T], BF)  # QT: head h at cols [h*T,(h+1)*T)
            kt_sb = persist.tile([128, KVPC * T], BF)  # KT per kv head
            vt_sb = persist.tile([128, KVPC * T], BF)  # VT per kv head
            v_sb = persist.tile([128, KVPC * T], BF)  # V[t,dv]: tile (g,kb) at (g*16+kb)*128

            # per-head AllGather buffers
            og_in = []
            og_out = []
            for h in range(HPC):
                oin = dram2.tile([128, T], BF, name=f"og_in_{h}")
                oout = dram2.tile([4 * 128, T], BF, name=f"og_out_{h}")
                og_in.append(oin)
                og_out.append(oout)

            with (
                tc.tile_pool(name="wall_pool", bufs=1) as wall_pool,
                tc.tile_pool(name="psum_p1", bufs=2, space="PSUM") as psum_p1,
            ):
                # phase-1 weights: c-tile cb at cols [cb*EALL, (cb+1)*EALL)
                wall_sb = wall_pool.tile([128, NC * EALL], BF)

                def load_wall(cb):
                    nc.sync.dma_start(
                        out=wall_sb[:, cb * EALL:(cb + 1) * EALL],
                        in_=wallT[cb * 128:(cb + 1) * 128, :],
                    )

                # ---------------- phase 1: projections ----------------
                # e-tile order: K0 K1 V0 V1 first so attention deps clear early
                etile_order = [HPC, HPC + 1, HPC + 2, HPC + 3] + list(range(HPC))

                def etile_dst(e):
                    # e indexes [Q0..Q7, K0, K1, V0, V1]
                    if e < HPC:
                        return qt_sb[:, e * T:(e + 1) * T]
                    if e < HPC + KVPC:
                        g = e - HPC
                        return kt_sb[:, g * T:(g + 1) * T]
                    g = e - HPC - KVPC
                    return vt_sb[:, g * T:(g + 1) * T]

                for chunk in range(3):  # 3 chunks of 4 e-tiles
                    es = etile_order[chunk * 4:(chunk + 1) * 4]
                    for tg in range(NT):
                        if chunk == 0 and tg == 0:
                            load_wall(0)
                            load_wall(1)
                            load_wall(2)
                        accs = []
                        for i, e in enumerate(es):
                            acc = psum_p1.tile([128, 512], F32, tag=f"acc{i}")
                            accs.append(acc)
                        for cb in range(NC):
                            if chunk == 0 and tg == 0 and cb + 3 < NC:
                                load_wall(cb + 3)
                            xt_t = stream.tile([128, 512], BF, tag="xt")
                            nc.sync.dma_start(
                                out=xt_t[:],
                                in_=xT[cb * 128:(cb + 1) * 128,
                                       tg * 512:(tg + 1) * 512],
                            )
                            for i, e in enumerate(es):
                                nc.tensor.matmul(
                                    accs[i][:],
                                    wall_sb[:, cb * EALL + e * 128:
                                            cb * EALL + (e + 1) * 128],
                                    xt_t[:],
                                    start=(cb == 0),
                                    stop=(cb == NC - 1),
                                )
                        for i, e in enumerate(es):
                            nc.vector.tensor_copy(
                                etile_dst(e)[:, tg * 512:(tg + 1) * 512], accs[i][:]
                            )

                # V = VT.T per 128x128 tile (PE transpose-mode; psum dtype = input)
                for g in range(KVPC):
                    for kb in range(NKB):
                        tp = psum_p1.tile([128, 128], BF, tag="acc0")
                        nc.tensor.transpose(
                            tp[:],
                            vt_sb[:, g * T + kb * 128:g * T + (kb + 1) * 128],
                            ident_sb[:],
                        )
                        nc.vector.tensor_copy(
                            v_sb[:, (g * NKB + kb) * 128:(g * NKB + kb + 1) * 128],
                            tp[:],
                        )

            # wall_pool/psum_p1 released; phase 2/3 reuse that SBUF/PSUM space.
            with (
                tc.tile_pool(name="p23", bufs=1) as p23,
                tc.tile_pool(name="work2", bufs=3) as work2,
            ):
                oT_sb = p23.tile([128, HPC * T], BF)  # local oT: head h at [h*T,..)
                woT_sb = p23.tile([128, NC * EQ], BF)  # phase-3 lhsT tiles
                for cb in range(NC):
                    nc.sync.dma_start(
                        out=woT_sb[:, cb * EQ:(cb + 1) * EQ],
                        in_=woT[cb * 128:(cb + 1) * 128, :],
                    )

                # ---------------- phase 2: attention ----------------
                # per-k-tile pipeline, 5-deep sT PSUM so the exp latency never
                # gates the PE; den via a gpsimd/DVE adder tree (quad sums) so
                # the PE only runs one accumulating den matmul per 4 k-tiles.
                # PSUM: sT 5x1 + oT 2x1 + den 1x1 = 8 banks.
                with (
                    tc.tile_pool(name="ps_sT", bufs=5, space="PSUM") as ps_sT,
                    tc.tile_pool(name="ps_oT", bufs=2, space="PSUM") as ps_oT,
                    tc.tile_pool(name="ps_den", bufs=1, space="PSUM") as ps_den,
                ):
                    for h in range(HPC):
                        g = h // R  # local kv head
                        qt_h = qt_sb[:, h * T:(h + 1) * T]
                        kt_g = kt_sb[:, g * T:(g + 1) * T]
                        for tg in range(NT):
                            nkb = 4 * tg + 4  # causal: k-tiles 0..nkb-1
                            nq = nkb // 4
                            oT_acc = ps_oT.tile([128, 512], F32, tag="oT")
                            den_acc = ps_den.tile([1, 512], F32, tag="den")
                            exps = []
                            for kb in range(nkb):
                                sT = ps_sT.tile([128, 512], F32, tag="sT")
                                nc.tensor.matmul(
                                    sT[:],
                                    kt_g[:, kb * 128:(kb + 1) * 128],
                                    qt_h[:, tg * 512:(tg + 1) * 512],
                                    start=True,
                                    stop=True,
                                )
                                jd = kb - 4 * tg  # diag 128-subtile (if 0..3)
                                jstart = max(0, jd)
                                if 0 <= jd < 4:
                                    nc.vector.tensor_tensor(
                                        sT[:, jd * 128:(jd + 1) * 128],
                                        sT[:, jd * 128:(jd + 1) * 128],
                                        mask_sb[:, 0:128],
                                        mybir.AluOpType.add,
                                    )
                                expT = work2.tile([128, 512], BF, tag="expT",
                                                  bufs=8)
                                if jstart > 0:
                                    nc.gpsimd.memset(expT[:, :jstart * 128], 0.0)
                                nc.scalar.activation(
                                    expT[:, jstart * 128:],
                                    sT[:, jstart * 128:],
                                    mybir.ActivationFunctionType.Exp,
                                    scale=INV_SQRT_HD,
                                )
                                exps.append(expT)
                                nc.tensor.matmul(
                                    oT_acc[:],
                                    v_sb[:, (g * NKB + kb) * 128:
                                         (g * NKB + kb + 1) * 128],
                                    expT[:],
                                    start=(kb == 0),
                                    stop=(kb == nkb - 1),
                                    skip_group_check=True,
                                )
                                if kb % 4 == 3:
                                    qd = kb // 4
                                    p0 = work2.tile([128, 512], BF, tag="dp0",
                                                    bufs=3)
                                    p1t = work2.tile([128, 512], BF, tag="dp1",
                                                     bufs=3)
                                    nc.vector.tensor_tensor(
                                        p0[:], exps[4 * qd][:],
                                        exps[4 * qd + 1][:],
                                        mybir.AluOpType.add,
                                    )
                                    nc.vector.tensor_tensor(
                                        p1t[:], exps[4 * qd + 2][:],
                                        exps[4 * qd + 3][:],
                                        mybir.AluOpType.add,
                                    )
                                    quad = work2.tile([128, 512], BF, tag="dq",
                                                      bufs=3)
                                    nc.vector.tensor_tensor(
                                        quad[:], p0[:], p1t[:],
                                        mybir.AluOpType.add,
                                    )
                                    nc.tensor.matmul(
                                        den_acc[:],
                                        ones_sb[:],
                                        quad[:],
                                        start=(qd == 0),
                                        stop=(qd == nq - 1),
                                        skip_group_check=True,
                                    )
                            recip = work2.tile([1, 512], F32, tag="recip", bufs=2)
                            nc.vector.reciprocal_approx_fast(
                                out=recip[:], in_=den_acc[:]
                            )
                            recip_b = work2.tile([128, 512], F32, tag="recip_b", bufs=2)
                            nc.gpsimd.partition_broadcast(recip_b[:], recip[:])
                            nc.vector.tensor_tensor(
                                oT_sb[:, h * T + tg * 512:h * T + (tg + 1) * 512],
                                oT_acc[:],
                                recip_b[:],
                                mybir.AluOpType.mult,
                            )
                        # ship this head's oT and gather peers'
                        nc.sync.dma_start(
                            out=og_in[h][:], in_=oT_sb[:, h * T:(h + 1) * T]
                        )
                        nc.gpsimd.collective_compute(
                            "AllGather",
                            mybir.AluOpType.bypass,
                            replica_groups=GROUPS,
                            ins=[og_in[h].opt()],
                            outs=[og_out[h].opt()],
                        )

                # ---------------- phase 3: outT slice = WoT.T @ oT_full --------
                # global e-tile eb <-> global head H: rank r = eb//8, local hl = eb%8
                with tc.tile_pool(name="ps_out", bufs=2, space="PSUM") as ps_out:
                    for tg in range(NT):
                        for occ in range(2):  # oc chunks of 4
                            accs = []
                            for oi in range(4):
                                acc = ps_out.tile([128, 512], F32, tag=f"out{oi}")
                                accs.append(acc)
                            eb_avail = [rr * HPC + hh
                                        for hh in range(HPC) for rr in range(4)]
                            for ei, eb in enumerate(eb_avail):  # 32 global e-tiles
                                r, hl = eb // HPC, eb % HPC
                                rhs_t = work2.tile([128, 512], BF, tag="rhs", bufs=8)
                                nc.sync.dma_start(
                                    out=rhs_t[:],
                                    in_=og_out[hl][r * 128:(r + 1) * 128,
                                                   tg * 512:(tg + 1) * 512],
                                )
                                for oi in range(4):
                                    oc = occ * 4 + oi
                                    nc.tensor.matmul(
                                        accs[oi][:],
                                        woT_sb[:, eb * EQ + oc * 128:
                                               eb * EQ + (oc + 1) * 128],
                                        rhs_t[:],
                                        start=(ei == 0),
                                        stop=(ei == NC - 1),
                                    )
                            for oi in range(4):
                                oc = occ * 4 + oi
                                ev = work2.tile([128, 512], F32, tag="ev")
                                nc.vector.tensor_copy(ev[:], accs[oi][:])
                                nc.sync.dma_start(
                                    out=out_part[oc * 128:(oc + 1) * 128,
                                                 tg * 512:(tg + 1) * 512],
                                    in_=ev[:],
                                )
    nc.finalize()
    return nc


_NC_CACHE = None


def _get_nc():
    global _NC_CACHE
    if _NC_CACHE is None:
        _NC_CACHE = build()
    return _NC_CACHE


def _make_maskL():
    """Packed left-aligned diag masks: for jd in 0..3 a [128, (jd+1)*128] block.

    Column c of block jd lies in subtile j = c // 128 (offset cc = c % 128);
    causal allow (k-offset r) <= (q-offset cc) on the diagonal subtile j == jd,
    full mask for j < jd, untouched (0) beyond the block width.
    """
    blocks = []
    r = np.arange(128)[:, None]
    for jd in range(4):
        w = (jd + 1) * 128
        c = np.arange(w)[None, :]
        j = c // 128
        cc = c % 128
        allow = (j == jd) & (r <= cc)
        m = np.where(allow, 0.0, -1e30).astype(np.float32)
        blocks.append(m)
    return np.concatenate(blocks, axis=1)  # [128, 1280]


def kernel(x, Wq, Wkv, Wo):
    x = np.asarray(x, dtype=np.float32)
    Wq = np.asarray(Wq, dtype=np.float32)
    Wkv = np.asarray(Wkv, dtype=np.float32)
    Wo = np.asarray(Wo, dtype=np.float32)

    # host-side prep (transposes + bf16 casts)
    try:
        import ml_dtypes

        bf16 = ml_dtypes.bfloat16
    except ImportError:  # pragma: no cover
        import jax.numpy as jnp

        bf16 = jnp.bfloat16

    xT_b = [np.ascontiguousarray(x[b].T).astype(bf16) for b in range(B)]

    maskL = _make_maskL()
    ident = np.eye(128, dtype=np.float32).astype(bf16)
    ones = np.ones((128, 1), dtype=np.float32).astype(bf16)

    in_maps = []
    for c in range(NCORES):
        b, j = c // 4, c % 4
        wq_l = Wq[EQ * j:EQ * (j + 1), :]  # [1024, 4096]
        wk_l = Wkv[EKV * j:EKV * (j + 1), :]  # [256, 4096]
        wv_l = Wkv[N_KV * HD + EKV * j:N_KV * HD + EKV * (j + 1), :]
        wall = np.concatenate([wq_l, wk_l, wv_l], axis=0)  # [1536, 4096]
        wallT = np.ascontiguousarray(wall.T).astype(bf16)  # [4096, 1536]
        woT_l = np.ascontiguousarray(Wo[EQ * j:EQ * (j + 1), :].T).astype(bf16)
        in_maps.append(
            {
                "xT": xT_b[b],
                "wallT": wallT,
                "woT": woT_l,
                "maskL": maskL,
                "ident": ident,
                "ones_in": ones,
            }
        )

    nc = _get_nc()
    res = run_bass_kernel_spmd(nc, in_maps, core_ids=list(range(NCORES)))

    out = np.empty((B, T, DIM), dtype=np.float32)
    for b in range(B):
        outT = np.concatenate(
            [res.results[b * 4 + j]["out_part"] for j in range(4)], axis=0
        )  # [4096, 2048]
        out[b] = outT.T
    return out
